# revision 9
# baseline (speedup 1.0000x reference)
"""Distributed 2-layer GAT (Article2Graph) on 8 TRN2 NeuronCores.

Math (per layer, reference):
    h = x @ W; s = h @ a_src; d = h @ a_dst
    e = leaky_relu(s_i + d_j, 0.01); masked by adj; attn = softmax_j; out = attn @ h

Key restructure used here (t = s_i + d_j):
    exp(lrelu(t)) = exp(0.01 t) * max(exp(0.99 t), 1)
                  = [exp(0.01 s_i)] * uj_j * max(vi_i * vj_j, 1)
  with uj_j = exp(0.01 d_j), vi_i = exp(0.99 s_i), vj_j = exp(0.99 d_j).
  The exp(0.01 s_i) factor is constant per row and cancels in the softmax.
  So with  w_ij = max(vi_i*vj_j, 1),  B_ij = adj_ij * w_ij:
    attn_ij = B_ij * uj_j / rowsum_i,   rowsum_i = sum_j B_ij * uj_j
    out_i   = (1/rowsum_i) * sum_j B_ij * (uj_j * h_j)   (fold uj into h)
  s and d are computed as x @ (W @ a_src) etc (associativity), which lets the
  per-node vectors come out of the same matmul that computes h.

Sharding: rows (dst nodes) split 1024/core; layer-1 features replicated;
one bf16 AllGather of layer-1 outputs between layers; mean-pool partials
combined on the host.
"""

import numpy as np
import ml_dtypes

N = 8192
F = 256
VOCAB = 50000
NCORES = 8
ROWS = N // NCORES          # 1024 rows per core
RB = ROWS // 128            # 8 row-blocks per core
KCH = N // 128              # 64 node chunks
HW = F + 2                  # h tile width: [h(256) | s | d]

_BUILD_CACHE = {}


def _build_nc(N=N, VOCAB=VOCAB):
    import concourse.bacc as bacc
    import concourse.bass as bass
    import concourse.mybir as mybir
    import concourse.tile as tile

    ROWS = N // NCORES
    RB = ROWS // 128
    KCH = N // 128

    BF16 = mybir.dt.bfloat16
    F32 = mybir.dt.float32
    I32 = mybir.dt.int32
    AF = mybir.ActivationFunctionType
    OP = mybir.AluOpType

    nc = bacc.Bacc("TRN2", num_devices=NCORES)

    # ---- I/O ----
    inDoc = nc.dram_tensor("inDoc", [N], I32, kind="ExternalInput")
    adj0 = nc.dram_tensor("adj0", [ROWS, N], BF16, kind="ExternalInput")
    adj1 = nc.dram_tensor("adj1", [ROWS, N], BF16, kind="ExternalInput")
    emb = nc.dram_tensor("emb", [VOCAB, F], F32, kind="ExternalInput")
    w1e = nc.dram_tensor("w1e", [F, HW], BF16, kind="ExternalInput")
    w2e = nc.dram_tensor("w2e", [F, HW], BF16, kind="ExternalInput")
    mask8 = nc.dram_tensor("mask8", [128, RB * KCH], F32, kind="ExternalInput")

    attn1_o = nc.dram_tensor("attn1", [ROWS, N], BF16, kind="ExternalOutput")
    attn2_o = nc.dram_tensor("attn2", [ROWS, N], BF16, kind="ExternalOutput")
    pool_o = nc.dram_tensor("pool", [1, F], F32, kind="ExternalOutput")

    with tile.TileContext(nc) as tc:
        with (
            tc.tile_pool(name="dram", bufs=1, space="DRAM") as dram,
            tc.tile_pool(name="big", bufs=2) as big,        # xT halves / B tiles
            tc.tile_pool(name="adjattn", bufs=2) as adjattn,
            tc.tile_pool(name="wbu", bufs=2) as wbu,
            tc.tile_pool(name="hpool", bufs=1) as hpool,
            tc.tile_pool(name="reppool", bufs=1) as reppool,
            tc.tile_pool(name="small", bufs=1) as small,
            tc.tile_pool(name="stage", bufs=4) as stage,
            tc.tile_pool(name="bt", bufs=4) as btpool,
            tc.tile_pool(name="ps", bufs=2, space="PSUM") as ps,
            tc.tile_pool(name="pst", bufs=3, space="PSUM") as pst,
            tc.tile_pool(name="psacc", bufs=2, space="PSUM") as psacc,
            tc.tile_pool(name="pssum", bufs=1, space="PSUM") as pssum,
        ):
            # ---------- constants ----------
            iota_p = small.tile([128, 1], I32, name="iota_p")
            iota_f = small.tile([128, 128], I32, name="iota_f")
            nc.gpsimd.iota(iota_p[:], pattern=[[0, 1]], base=0, channel_multiplier=1)
            nc.gpsimd.iota(iota_f[:], pattern=[[1, 128]], base=0, channel_multiplier=0)
            iota_pf = small.tile([128, 1], F32, name="iota_pf")
            iota_ff = small.tile([128, 128], F32, name="iota_ff")
            nc.vector.tensor_copy(iota_pf[:], iota_p[:])
            nc.vector.tensor_copy(iota_ff[:], iota_f[:])
            ident_f32 = small.tile([128, 128], F32, name="ident_f32")
            ident_bf = small.tile([128, 128], BF16, name="ident_bf")
            nc.vector.tensor_tensor(
                out=ident_f32[:], in0=iota_ff[:],
                in1=iota_pf[:].to_broadcast([128, 128]), op=OP.is_equal)
            nc.scalar.copy(ident_bf[:], ident_f32[:])
            ones_row = small.tile([1, 128], BF16, name="ones_row")
            nc.vector.memset(ones_row[:], 1.0)
            ones_col = small.tile([128, 1], F32, name="ones_col")
            nc.vector.memset(ones_col[:], 1.0)

            mask_sb = small.tile([128, RB * KCH], F32, name="mask_sb")
            nc.sync.dma_start(mask_sb[:], mask8[:])

            # weight tiles: [128, HW] x 2 halves per layer
            wt = []
            for li, wsrc in enumerate((w1e, w2e)):
                w_t = small.tile([128, 2 * HW], BF16, name=f"w_t{li}")
                nc.sync.dma_start(w_t[:, 0:HW], wsrc[0:128, :])
                nc.sync.dma_start(w_t[:, HW:2 * HW], wsrc[128:256, :])
                wt.append(w_t)

            # index tile for the embedding gather: idx[p, k] = inDoc[k*128+p]
            idx_sb = small.tile([128, KCH], I32, name="idx_sb")
            nc.sync.dma_start(
                idx_sb[:], inDoc.rearrange("(k p) -> p k", p=128))

            out1_f32 = small.tile([128, RB * F], F32, name="out1_f32")
            agin = dram.tile([ROWS, F], BF16, name="agin")
            agout = dram.tile([N, F], BF16, name="agout", addr_space="Shared")
            dtmp = dram.tile([N], BF16, name="dtmp", bufs=2)

            sum_ps = pssum.tile([1, F], F32, name="sum_ps")

            def layer_prep(li):
                """Build xT (transposed features), h tile (with s,d cols),
                rep tensors VJ/UJ, uj in p-major, per-rb vi and h~ = uj*h."""
                src_f32 = li == 0
                dt_in = F32 if src_f32 else BF16
                ident = ident_f32 if src_f32 else ident_bf

                xT0 = big.tile([128, N], BF16, name=f"xT0_{li}", tag="big")
                xT1 = big.tile([128, N], BF16, name=f"xT1_{li}", tag="big")
                # gather/load 128-node chunks, transpose to xT halves
                for g in range(KCH // 4):
                    tp0 = pst.tile([128, 512], dt_in, name=f"tp0_{li}_{g}", tag="pst")
                    tp1 = pst.tile([128, 512], dt_in, name=f"tp1_{li}_{g}", tag="pst")
                    for t in range(4):
                        k = 4 * g + t
                        xch = stage.tile([128, F], dt_in, name=f"xch_{li}_{k}",
                                         tag="xch")
                        if src_f32:
                            nc.gpsimd.indirect_dma_start(
                                out=xch[:], out_offset=None, in_=emb[:],
                                in_offset=bass.IndirectOffsetOnAxis(
                                    ap=idx_sb[:, k:k + 1], axis=0))
                        else:
                            nc.sync.dma_start(
                                xch[:], agout[k * 128:(k + 1) * 128, :])
                        nc.tensor.transpose(
                            tp0[:, t * 128:(t + 1) * 128], xch[:, 0:128], ident[:])
                        nc.tensor.transpose(
                            tp1[:, t * 128:(t + 1) * 128], xch[:, 128:256], ident[:])
                    nc.scalar.copy(xT0[:, g * 512:(g + 1) * 512], tp0[:])
                    nc.scalar.copy(xT1[:, g * 512:(g + 1) * 512], tp1[:])

                # h chunks: [h | s | d] = xT_chunk.T @ [W | a~ | b~]
                h_t = hpool.tile([128, KCH * HW], BF16, name=f"h_{li}", tag="h")
                for k in range(KCH):
                    hp = ps.tile([128, HW], F32, name=f"hp_{li}_{k}", tag="hp")
                    nc.tensor.matmul(
                        hp[:], xT0[:, k * 128:(k + 1) * 128], wt[li][:, 0:HW],
                        start=True, stop=False)
                    nc.tensor.matmul(
                        hp[:], xT1[:, k * 128:(k + 1) * 128], wt[li][:, HW:2 * HW],
                        start=False, stop=True)
                    nc.scalar.copy(h_t[:, k * HW:(k + 1) * HW], hp[:])

                # strided views of the s and d columns (p-major layout)
                s_pm = h_t[:, F::HW]       # [128, KCH] bf16, s_pm[p,k]=s[k*128+p]
                d_pm = h_t[:, F + 1::HW]

                # uj (p-major) for h~ scaling
                uj_pm = small.tile([128, KCH], F32, name=f"uj_pm_{li}", tag="ujpm")
                nc.scalar.activation(uj_pm[:], d_pm, AF.Exp, scale=0.01)
                for k in range(KCH):
                    nc.vector.tensor_scalar_mul(
                        h_t[:, k * HW:k * HW + F], h_t[:, k * HW:k * HW + F],
                        uj_pm[:, k:k + 1])

                # d to a single row, then broadcast via ones-matmul
                nc.sync.dma_start(dtmp[:].rearrange("(k p) -> p k", p=128), d_pm)
                drow2 = dtmp[:].rearrange("(g n) -> g n", n=512)
                vj_rep = reppool.tile([128, N], BF16, name=f"vj_{li}", tag="vj")
                uj_rep = reppool.tile([128, N], BF16, name=f"uj_{li}", tag="uj")
                for g in range(N // 512):
                    drg = small.tile([1, 512], BF16, name=f"drg_{li}_{g}",
                                     tag="drg", bufs=2)
                    nc.sync.dma_start(drg[:], drow2[g:g + 1, :])
                    dp = pst.tile([128, 512], F32, name=f"dp_{li}_{g}", tag="pst")
                    nc.tensor.matmul(
                        dp[:], ones_row[:], drg[:],
                        start=True, stop=True)
                    nc.scalar.activation(
                        vj_rep[:, g * 512:(g + 1) * 512], dp[:], AF.Exp, scale=0.99)
                    nc.scalar.activation(
                        uj_rep[:, g * 512:(g + 1) * 512], dp[:], AF.Exp, scale=0.01)

                # per-row-block vi = exp(0.99 * s_own)
                vi_t = small.tile([128, RB], F32, name=f"vi_{li}", tag="vi")
                sjunk = small.tile([128, KCH], F32, name=f"sjunk_{li}", tag="sjunk")
                sown = small.tile([128, RB], F32, name=f"sown_{li}", tag="sown")
                for rb in range(RB):
                    nc.vector.tensor_tensor(
                        out=sjunk[:], in0=s_pm,
                        in1=mask_sb[:, rb * KCH:(rb + 1) * KCH], op=OP.mult)
                    nc.vector.reduce_sum(
                        out=sown[:, rb:rb + 1], in_=sjunk[:],
                        axis=mybir.AxisListType.X)
                    nc.scalar.activation(
                        vi_t[:, rb:rb + 1], sown[:, rb:rb + 1], AF.Exp, scale=0.99)
                # s column is dead now; overwrite it with uj so the matmul's
                # 257th column accumulates the softmax row-sum for free
                nc.vector.tensor_copy(h_t[:, F::HW], uj_pm[:])
                return h_t, vj_rep, uj_rep, vi_t

            def layer_rows(li, h_t, vj_rep, uj_rep, vi_t, adj_in, attn_out):
                """Row-block loop: returns out rows (f32) written to out1_f32
                (li=0) or doc accumulation (li=1)."""
                for rb in range(RB):
                    adj_t = adjattn.tile([128, N], BF16, name=f"adj_{li}_{rb}",
                                         tag="aa")
                    nc.sync.dma_start(
                        adj_t[:], adj_in[rb * 128:(rb + 1) * 128, :])
                    w_t = wbu.tile([128, N], BF16, name=f"w_{li}_{rb}", tag="wbu")
                    nc.vector.tensor_scalar(
                        out=w_t[:], in0=vj_rep[:], scalar1=vi_t[:, rb:rb + 1],
                        scalar2=1.0, op0=OP.mult, op1=OP.max)
                    b_t = big.tile([128, N], BF16, name=f"b_{li}_{rb}", tag="big")
                    nc.vector.tensor_tensor(
                        out=b_t[:], in0=adj_t[:], in1=w_t[:], op=OP.mult)
                    bu_t = wbu.tile([128, N], BF16, name=f"bu_{li}_{rb}", tag="wbu")
                    nc.vector.tensor_tensor(
                        out=bu_t[:], in0=b_t[:], in1=uj_rep[:], op=OP.mult)
                    # out rows: acc = sum_j B^T_chunk.T @ h~_chunk
                    acc = psacc.tile([128, F + 1], F32, name=f"acc_{li}_{rb}",
                                     tag="acc")
                    for g in range(KCH // 4):
                        tp = pst.tile([128, 512], BF16, name=f"tp_{li}_{rb}_{g}",
                                      tag="pst")
                        for t in range(4):
                            k = 4 * g + t
                            nc.tensor.transpose(
                                tp[:, t * 128:(t + 1) * 128],
                                b_t[:, k * 128:(k + 1) * 128], ident_bf[:])
                        bts = btpool.tile([128, 512], BF16,
                                          name=f"bts_{li}_{rb}_{g}", tag="bts")
                        nc.scalar.copy(bts[:], tp[:])
                        for t in range(4):
                            k = 4 * g + t
                            nc.tensor.matmul(
                                acc[:], bts[:, t * 128:(t + 1) * 128],
                                h_t[:, k * HW:k * HW + F + 1],
                                start=(k == 0), stop=(k == KCH - 1))
                    recip = small.tile([128, 1], F32, name=f"rc_{li}_{rb}",
                                       tag="rc", bufs=2)
                    nc.vector.reciprocal(recip[:], acc[:, F:F + 1])
                    attn_t = wbu.tile([128, N], BF16, name=f"at_{li}_{rb}",
                                      tag="wbu")
                    nc.vector.tensor_scalar_mul(attn_t[:], bu_t[:], recip[:])
                    nc.sync.dma_start(
                        attn_out[rb * 128:(rb + 1) * 128, :], attn_t[:])
                    if li == 0:
                        nc.vector.tensor_scalar_mul(
                            out1_f32[:, rb * F:(rb + 1) * F], acc[:, 0:F],
                            recip[:])
                        o1b = stage.tile([128, F], BF16, name=f"o1b_{rb}",
                                         tag="o1b", bufs=2)
                        nc.scalar.copy(o1b[:], out1_f32[:, rb * F:(rb + 1) * F])
                        nc.sync.dma_start(
                            agin[rb * 128:(rb + 1) * 128, :], o1b[:])
                    else:
                        dt = stage.tile([128, F], F32, name=f"doc_{rb}", tag="doc",
                                        bufs=2)
                        nc.vector.tensor_scalar_mul(dt[:], acc[:, 0:F], recip[:])
                        nc.vector.tensor_tensor(
                            out=dt[:], in0=dt[:],
                            in1=out1_f32[:, rb * F:(rb + 1) * F], op=OP.add)
                        nc.tensor.matmul(
                            sum_ps[:], ones_col[:], dt[:],
                            start=(rb == 0), stop=(rb == RB - 1))

            # ---------- layer 1 ----------
            h1, vj1, uj1, vi1 = layer_prep(0)
            layer_rows(0, h1, vj1, uj1, vi1, adj0, attn1_o)

            # ---------- AllGather ----------
            nc.gpsimd.collective_compute(
                "AllGather", mybir.AluOpType.bypass,
                replica_groups=[list(range(NCORES))],
                ins=[agin[:]], outs=[agout[:]])

            # ---------- layer 2 ----------
            h2, vj2, uj2, vi2 = layer_prep(1)
            layer_rows(1, h2, vj2, uj2, vi2, adj1, attn2_o)

            # ---------- pool partial ----------
            sum_sb = small.tile([1, F], F32, name="sum_sb")
            nc.vector.tensor_copy(sum_sb[:], sum_ps[:])
            nc.sync.dma_start(pool_o[:], sum_sb[:])

    nc.compile()
    return nc


def _get_nc():
    if "nc" not in _BUILD_CACHE:
        _BUILD_CACHE["nc"] = _build_nc()
    return _BUILD_CACHE["nc"]


def prepare_in_maps(inDoc, adj0, adj1, emb, W1, a1_src, a1_dst, W2, a2_src,
                    a2_dst, n=N):
    ROWS = n // NCORES
    RB = ROWS // 128
    KCH = n // 128
    inDoc = np.asarray(inDoc).astype(np.int32)
    emb = np.ascontiguousarray(np.asarray(emb, dtype=np.float32))
    bf = ml_dtypes.bfloat16
    adj0b = np.asarray(adj0, dtype=np.float32).astype(bf)
    adj1b = np.asarray(adj1, dtype=np.float32).astype(bf)

    def wext(W, asrc, adst):
        W = np.asarray(W, dtype=np.float32)
        at = W @ np.asarray(asrc, dtype=np.float32)
        bt = W @ np.asarray(adst, dtype=np.float32)
        return np.concatenate(
            [W, at[:, None], bt[:, None]], axis=1).astype(bf)

    w1e = wext(W1, a1_src, a1_dst)
    w2e = wext(W2, a2_src, a2_dst)

    in_maps = []
    for c in range(NCORES):
        m8 = np.zeros((128, RB * KCH), dtype=np.float32)
        for rb in range(RB):
            m8[:, rb * KCH + c * RB + rb] = 1.0
        in_maps.append({
            "inDoc": inDoc,
            "adj0": adj0b[c * ROWS:(c + 1) * ROWS],
            "adj1": adj1b[c * ROWS:(c + 1) * ROWS],
            "emb": emb,
            "w1e": w1e, "w2e": w2e,
            "mask8": m8,
        })
    return in_maps


def kernel(inDoc, adj0, adj1, emb, W1, a1_src, a1_dst, W2, a2_src, a2_dst):
    from concourse.bass_utils import run_bass_kernel_spmd

    in_maps = prepare_in_maps(inDoc, adj0, adj1, emb, W1, a1_src, a1_dst,
                              W2, a2_src, a2_dst)
    nc = _get_nc()
    res = run_bass_kernel_spmd(nc, in_maps, core_ids=list(range(NCORES)),
                               trace=False)
    outs = res.results
    satt = np.concatenate(
        [outs[c]["attn1"].astype(np.float32) for c in range(NCORES)], axis=0)
    datt = np.concatenate(
        [outs[c]["attn2"].astype(np.float32) for c in range(NCORES)], axis=0)
    pool = sum(outs[c]["pool"][0].astype(np.float32) for c in range(NCORES))
    doc_mean = (pool / N).astype(np.float32)
    return doc_mean, satt, datt
